# revision 4
# baseline (speedup 1.0000x reference)
"""Trainium2 Bass kernel for nn_HermesMessageLayer (gnn_message_passing).

Math: out[e,i,n] = sum_{b,f,r,j,m} inp[e,j,m] * precomp[e,f,r]
                                   * kernel[b,f,n,m] * weight[b,r,i,j] + bias[i]

Staging (per core, data-parallel over E across 8 cores):
  KW[(j,m), (ni, fr)] = sum_b kernel[b,f,n,m]*weight[b,r,i,j]   (host, tiny;
                        columns ordered ni-major / fr-innermost)
  t[e, ni, fr] = inp[e,(j,m)] @ KW                              (PE matmul)
  out[e, ni]   = sum_fr pc[e,fr] * t[e,ni,fr]                   (one custom
                 DVE mul-cumsum op per tile + a Pool strided diff)
  bias is added on the host during unpack.

Per 128-edge tile on device:
  - inp rows transpose-loaded (XBAR DMA, bf16, rows padded 96->128) so the
    contraction dim (j,m) lands on SBUF partitions for the matmul stationary.
  - one PE matmul pair (N=480 x2, two PSUM banks) computes t for 128 edges.
  - one custom DVE op (MUL_CUMSUM_ANT: scan(ADD, Src0*Src1)) reads t straight
    from PSUM (fp32) and pc via a stride-0 broadcast AP, writing the running
    per-(ni,fr) product cumsum S to SBUF in fp32.
  - Pool (gpsimd) computes the per-page sums acc[ni] = S[10(ni+1)] - S[10 ni]
    with strided APs (leading S column memset to 0), cast to bf16.
  - bf16 HWDGE store to a partition-major layout; host un-permutes + upcasts
    and adds bias.
"""

import sys

import numpy as np

sys.path.insert(0, "/opt/trn_rl_repo")

import ml_dtypes

import concourse.bass as bass
import concourse.bacc as bacc
import concourse.tile as tile
from concourse import mybir
from concourse.bass_utils import run_bass_kernel_spmd

# ---- custom DVE op: out[k] = cumsum_k(in0[k] * in1[k]) --------------------
from concourse import dve_ops
from concourse.dve_spec import Spec, Src0, Src1, scan, AluOp, lower
from concourse.dve_uop import DveOpSpec


def _mul_cumsum_ref(in0, in1, s0, s1, imm2):
    p = in0.shape[0]
    prod = in0.astype(np.float32).reshape(p, -1) * in1.astype(np.float32).reshape(
        p, -1
    )
    return np.cumsum(prod, axis=1)


_MUL_CUMSUM_SPEC = Spec(body=scan(AluOp.ADD, Src0 * Src1), reference=_mul_cumsum_ref)
_OP_NAME = "MUL_CUMSUM_ANT"


def _register_mul_cumsum():
    if _OP_NAME in dve_ops._SUB_OPCODE_FOR_NAME:
        return next(o for o in dve_ops.OPS if o.name == _OP_NAME)
    row = dve_ops._CUSTOM_DVE_ROW_BASE + len(dve_ops.OPS)
    shas = {
        ver: DveOpSpec(
            name=_OP_NAME, opcode=row, uops=lower(_MUL_CUMSUM_SPEC, ver=ver), rd1_en=True
        ).sha(ver)
        for ver in ("v3", "v4")
    }
    op = dve_ops.DveOp(_OP_NAME, _MUL_CUMSUM_SPEC, subdim=False, uops_sha=shas)
    dve_ops.OPS.append(op)
    dve_ops._SUB_OPCODE_FOR_NAME[_OP_NAME] = row
    dve_ops.CUSTOM_DVE_SPECS[_OP_NAME] = _MUL_CUMSUM_SPEC
    return op


MUL_CUMSUM = _register_mul_cumsum()

# Problem dims
E, J, I = 300000, 32, 32
M, N = 3, 3
B, F, R = 6, 5, 2
JM = J * M          # 96
NI = I * N          # 96  (col layout is (i, n): ni = i*3 + n)
FR = F * R          # 10
TCOLS = FR * NI     # 960

NCORES = 8
E_CORE = E // NCORES            # 37500
G = 16                          # tiles per group
TILE_E = 128                    # edges per tile (PSUM partitions)
GROUP_E = G * TILE_E            # 2048
NG = -(-E_CORE // GROUP_E)      # 19 groups
E_PAD = NG * GROUP_E            # 38912

BF16 = mybir.dt.bfloat16
F32 = mybir.dt.float32


N_ACT = 5                       # tiles per group handled by the ScalarE+Pool
                                # pipeline (the rest go through the DVE scan)
WARM_MM = 12                    # back-to-back warmup matmuls (~4.8 us cold)


def build_program(ng: int = NG, n_act: int = N_ACT):
    """Build the single-core Bass program (same program runs SPMD on all cores)."""
    nc = bacc.Bacc("TRN2", target_bir_lowering=False, debug=False)

    e_pad = ng * GROUP_E
    inp_t = nc.dram_tensor("inp_aug", [e_pad, 128], BF16, kind="ExternalInput").ap()
    pc_t = nc.dram_tensor("pc", [ng, 128, G, FR], F32, kind="ExternalInput").ap()
    kw_t = nc.dram_tensor("kw", [JM, TCOLS], BF16, kind="ExternalInput").ap()
    out_t = nc.dram_tensor("out", [ng, 128, G, NI], BF16, kind="ExternalOutput").ap()

    with tile.TileContext(nc) as tc:
        with (
            tc.tile_pool(name="const", bufs=1) as const_pool,
            tc.tile_pool(name="inpT", bufs=2) as inpT_pool,
            tc.tile_pool(name="pc", bufs=2) as pc_pool,
            tc.tile_pool(name="scan", bufs=3) as scan_pool,
            tc.tile_pool(name="u", bufs=2) as u_pool,
            tc.tile_pool(name="w", bufs=2) as w_pool,
            tc.tile_pool(name="acc", bufs=2) as acc_pool,
            tc.tile_pool(name="psum", bufs=3, space="PSUM") as psum_pool,
            tc.tile_pool(name="warm", bufs=1, space="PSUM") as warm_pool,
        ):
            kw_sb = const_pool.tile([JM, TCOLS], BF16)
            nc.sync.dma_start(kw_sb[:], kw_t[:])

            # HAM warmup: ~5 us of contiguous PE activity releases the clock
            # gate (K=4/8 -> 8/8) before the steady-state loop begins.
            warm = warm_pool.tile([128, 512], F32)
            for _ in range(WARM_MM):
                nc.tensor.matmul(
                    warm[:, 0:480],
                    kw_sb[:, 0:128],
                    kw_sb[:, 0:480],
                    start=True,
                    stop=True,
                )

            for g in range(ng):
                inpT = inpT_pool.tile([128, GROUP_E], BF16)
                nc.sync.dma_start(
                    inpT[:],
                    inp_t[g * GROUP_E : (g + 1) * GROUP_E, :],
                    transpose=True,
                )
                pc = pc_pool.tile([128, G, FR], F32)
                nc.sync.dma_start(pc[:], pc_t[g])
                acc = acc_pool.tile([128, G, NI], BF16)

                for gi in range(G):
                    ps = psum_pool.tile([128, 1024], F32)
                    lhsT = inpT[0:JM, gi * TILE_E : (gi + 1) * TILE_E]
                    nc.tensor.matmul(
                        ps[:, 0:480], lhsT, kw_sb[:, 0:480], start=True, stop=True
                    )
                    nc.tensor.matmul(
                        ps[:, 512:992], lhsT, kw_sb[:, 480:960], start=True, stop=True
                    )
                    ps_b = ps[:].rearrange("p (b x) -> p b x", b=2)

                    if gi >= G - n_act:
                        # ScalarE+Pool pipeline: 10 per-partition-scaled
                        # copies (ACT, PSUM-direct) + a Pool add tree.
                        u = u_pool.tile([128, FR, NI], BF16)
                        for fr in range(FR):
                            nc.scalar.mul(
                                u[:, fr],
                                ps_b[:, :, fr:480:FR],
                                pc[:, gi, fr : fr + 1],
                            )
                        w = w_pool.tile([128, 6, NI], BF16)
                        nc.gpsimd.tensor_add(w[:, 0:5], u[:, 0:5], u[:, 5:10])
                        nc.gpsimd.tensor_add(w[:, 5], w[:, 0], w[:, 1])
                        nc.gpsimd.tensor_add(w[:, 5], w[:, 5], w[:, 2])
                        nc.gpsimd.tensor_add(w[:, 5], w[:, 5], w[:, 3])
                        nc.gpsimd.tensor_add(acc[:, gi], w[:, 5], w[:, 4])
                    else:
                        # DVE pipeline: one fused mul-cumsum over (ni, fr)
                        # PSUM-direct, then a Pool strided page-diff.
                        s = scan_pool.tile([128, TCOLS + 1], F32)
                        nc.gpsimd.memset(s[:, 0:1], 0.0)
                        pc_b = (
                            pc[:, gi]
                            .rearrange("p (o fr) -> p o fr", o=1)
                            .broadcast_to([128, NI, FR])
                        )
                        nc.vector._custom_dve(
                            MUL_CUMSUM,
                            out=s[:, 1 : TCOLS + 1],
                            in0=ps_b[:, :, 0:480],
                            in1=pc_b,
                        )
                        # acc[ni] = S[10(ni+1)] - S[10 ni]: per-ni page sums
                        nc.gpsimd.tensor_sub(
                            acc[:, gi],
                            s[:, FR : TCOLS + 1 : FR],
                            s[:, 0:TCOLS:FR],
                        )

                nc.sync.dma_start(out_t[g], acc[:])

    nc.compile()
    return nc


def _pack_core(inp_c, precomp_c, ng: int = NG):
    """Pack one core's slice into the padded/permuted device layouts."""
    e_pad = ng * GROUP_E
    e_c = inp_c.shape[0]
    inp_aug = np.zeros([e_pad, 128], dtype=ml_dtypes.bfloat16)
    inp_aug[:e_c, :JM] = inp_c.reshape(e_c, JM).astype(ml_dtypes.bfloat16)

    pc_pad = np.zeros([e_pad, FR], dtype=np.float32)
    pc_pad[:e_c] = precomp_c.reshape(e_c, FR)
    # tile (g, gi) partition p holds edge g*GROUP_E + gi*TILE_E + p
    pc_perm = np.ascontiguousarray(
        pc_pad.reshape(ng, G, TILE_E, FR).transpose(0, 2, 1, 3)
    )
    return inp_aug, pc_perm


def _pack_shared(kernel, weight):
    # KW[(j,m), (i,n,f,r)] = sum_b kernel[b,f,n,m] * weight[b,r,i,j]
    # column order: ni-major, fr-innermost  (col = ni*FR + fr)
    kw = np.einsum(
        "bfnm,brij->jminfr",
        kernel.astype(np.float64),
        weight.astype(np.float64),
    ).reshape(JM, TCOLS)
    return kw.astype(ml_dtypes.bfloat16)


_PROGRAM_CACHE = {}


def _get_program(ng: int = NG, n_act: int = N_ACT):
    key = (ng, n_act)
    if key not in _PROGRAM_CACHE:
        _PROGRAM_CACHE[key] = build_program(ng, n_act)
    return _PROGRAM_CACHE[key]


def kernel(inp, precomp, kernel, weight, bias):
    inp = np.asarray(inp)
    precomp = np.asarray(precomp)
    kernel_np = np.asarray(kernel)
    weight = np.asarray(weight)
    bias = np.asarray(bias)

    kw_b = _pack_shared(kernel_np, weight)

    in_maps = []
    for c in range(NCORES):
        sl = slice(c * E_CORE, (c + 1) * E_CORE)
        inp_aug, pc_perm = _pack_core(inp[sl], precomp[sl])
        in_maps.append({"inp_aug": inp_aug, "pc": pc_perm, "kw": kw_b})

    nc = _get_program()
    res = run_bass_kernel_spmd(nc, in_maps, list(range(NCORES)))

    out = np.empty([E, I, N], dtype=np.float32)
    for c in range(NCORES):
        o = np.asarray(res.results[c]["out"]).astype(np.float32)  # [NG,128,G,NI]
        o = o.transpose(0, 2, 1, 3).reshape(NG * GROUP_E, NI)[:E_CORE]
        out[c * E_CORE : (c + 1) * E_CORE] = o.reshape(E_CORE, I, N)
    out += bias.astype(np.float32)[None, :, None]
    return out
